# revision 18
# baseline (speedup 1.0000x reference)
"""MemNN (end-to-end memory network) Trainium2 kernel.

The heavy FLOPs are six (B*L, V) @ (V, D) embedding matmuls sharing `facts`
as LHS (A_h = facts @ Wa[h], C_h = facts @ Wc[h]) that fuse into one
(3200, 10000) @ (10000, 1280) matmul independent of the hop recurrence.
Sharding: vocab (contraction) split 8 ways; each core computes a partial
product, the host sums the 8 bf16 partials and runs the tiny hop recurrence.

The entire fused matmul runs in fp8 e4m3 with MatmulPerfMode.DoubleRow:
measured on this hardware, DoubleRow streams 2 pair-rows/cycle (4 effective
moving rows/cycle, 2x bf16) and covers two 128-row contraction tiles per
weight load.  One (V, 1280) weight table [Wa1|Wa2|Wc0|Wc1|Wc2], one shifted
fp8 facts tensor, 50 identical DoubleRow instructions per moving chunk.

Precision engineering against the 2e-2 tolerance (measured 1.28e-2):
 - facts are mean-shifted (f - 0.5) before quantizing, halving fp8 noise;
   the exact rank-1 corrections are applied on the host
   (match_h += 0.5*(u . colsum(Wa_h)); att_h += 0.5*colsum(Wc_h), a
   constant add since p sums to 1).  Weights are pre-scaled by 2^11 so
   their ~0.02 entries land in e4m3's normal range (max finite 240).
 - hop 0, whose attention logits have the largest spread and dominate the
   flip risk, is EXACT: u_0 = sum(question) @ Wq and
   match_0[b,l] = facts[b,l,:] . (Wa0 @ u_0[b]) are computed on the host
   in fp32 -- 0.7% of the FLOPs by associativity (the rank-64 projection
   avoids the (B*L, V) @ (V, D) hop-0 matmul entirely).  With hop 0 exact,
   single-term fp8 suffices for A_1/A_2 (1.3e-2 vs 2.4e-2 otherwise).
 - Partials are written in bf16 (halves output DMA; adds ~1e-3 noise).

Moving chunks are 512 wide (the PSUM-bank limit), minimizing instruction
and weight-load count per streamed row.
"""

import os

os.environ.setdefault("MYCRO_LOCAL_CACHE", "1")

import ml_dtypes
import numpy as np

import concourse.bass as bass
import concourse.mybir as mybir
import concourse.tile as tile
from concourse.bass_utils import run_bass_kernel_spmd

HOPS, B, L, V, D = 3, 64, 50, 10000, 256
NCORES = 8
BL = B * L                # 3200 moving rows
NA = 2 * D                # 512 A cols: [Wa1|Wa2]
NC_ = HOPS * D            # 768 C cols: [Wc0|Wc1|Wc2]
NW = NA + NC_             # 1280 fused fp8 weight cols: [Wa1|Wa2|Wc0|Wc1|Wc2]
VSH = V // NCORES         # 1250 vocab rows per core
KT = 10                   # contraction tiles of 128 per core
VPAD = KT * 128           # 1280 (zero-padded)
MCH = 400                 # moving-col chunk
WSC = 2048.0              # 2^11 Wc pre-scale for fp8
F32 = mybir.dt.float32
BF16 = mybir.dt.bfloat16
FP8 = mybir.dt.float8e4
NP_BF16 = ml_dtypes.bfloat16
NP_FP8 = ml_dtypes.float8_e4m3
DR = mybir.MatmulPerfMode.DoubleRow

_nc_cache = None
_last_result = None       # BassKernelResults of the most recent run (for profiling)


def _legalize_sync(nc):
    """Split multi-wait sync_info into standalone single-wait EventSemaphores.

    The walrus build in this environment enforces the raw-bass contract of at
    most ONE SyncWait per instruction ("Too many sync wait commands" in
    setupSyncWait otherwise), while Tile attaches every needed wait to the
    consuming instruction.  Hoisting all-but-one wait onto preceding
    InstEventSemaphore instructions on the same engine queue is semantically
    identical: engine queues are in-order, so a preceding wait blocks the
    queue exactly like an attached wait.  Updates are left untouched (they
    fire at completion and cannot be hoisted).
    """
    for func in nc.m.functions:
        for block in func.blocks:
            insts = list(block.instructions)
            out = []
            n = 0
            for inst in insts:
                si = inst.sync_info
                if si is not None and len(si.on_wait) > 1:
                    waits = list(si.on_wait)
                    for w in waits[:-1]:
                        ev = mybir.InstEventSemaphore(
                            name=f"{inst.name}-hoistw{n}", ins=[], outs=[]
                        )
                        n += 1
                        ev.engine = inst.engine
                        ev.sync_info = mybir.SyncInfo(on_wait=[w], on_update=[])
                        nc.register_instruction(ev)
                        out.append(ev)
                    inst.sync_info = mybir.SyncInfo(
                        on_wait=[waits[-1]], on_update=list(si.on_update)
                    )
                out.append(inst)
            if len(out) != len(insts):
                block.instructions = out
    return nc


# 512-wide chunks (the PSUM-bank / moving-AP limit) minimize instruction
# and weight-load count per streamed row; the 128 tail carries the rest.
_WIDTHS = [512] * 6 + [128]
_STARTS = [sum(_WIDTHS[:i]) for i in range(len(_WIDTHS))]
assert sum(_WIDTHS) == BL


def _build(reps=1):
    """Build the SPMD device program.

    reps>1 repeats the main loop body (same data, same output addresses) --
    used only by the benchmark harness to measure device time differentially
    (per-call dispatch noise over the axon tunnel is ~ms, device time is
    ~100 us, so wall-clocking one launch cannot resolve it).
    """
    nc = bass.Bass(trn_type="TRN2")
    facts_8 = nc.dram_tensor("facts_8", [VPAD, BL], FP8, kind="ExternalInput")
    w_8 = nc.dram_tensor("w_8", [VPAD, NW], FP8, kind="ExternalInput")
    pac_b = nc.dram_tensor("pac_b", [NW, BL], BF16, kind="ExternalOutput")

    f8r = facts_8.rearrange("(k p) n -> p k n", p=128)
    wr = w_8.rearrange("(k p) n -> p k n", p=128)
    wmax = max(_WIDTHS)
    NNW = NW // 128           # 10 fp8 DoubleRow n-tiles

    with (
        tile.TileContext(nc) as tc,
        tc.tile_pool(name="wpool", bufs=1) as wpool,
        tc.tile_pool(name="x8pool", bufs=3) as x8pool,
        tc.tile_pool(name="opool", bufs=6) as opool,
        tc.tile_pool(name="pspool", bufs=7, space="PSUM") as pspool,
    ):
        x8s = {}

        def get_xt(mi, xs, pool, dt, rr, tg):
            if mi not in xs:
                xs[mi] = pool.tile(
                    [128, KT, _WIDTHS[mi]], dt, tag=tg, name=tg,
                    padded_shape=[128, KT, wmax],
                )
                nc.sync.dma_start(
                    xs[mi][:], rr[:, :, _STARTS[mi] : _STARTS[mi] + _WIDTHS[mi]]
                )
            return xs[mi]

        def drain(ps, dst, row0, nrows, mi):
            ot = opool.tile(
                [nrows, _WIDTHS[mi]], BF16, tag="ot", name="ot",
                padded_shape=[128, wmax],
            )
            nc.vector.tensor_copy(ot[:], ps[:])
            nc.sync.dma_start(
                dst[row0 : row0 + nrows, _STARTS[mi] : _STARTS[mi] + _WIDTHS[mi]],
                ot[:],
            )

        # Main fused matmul per chunk: the 64-wide match0 group, 4 bf16 A
        # n-tiles, then 6 fp8 DoubleRow C n-tiles.  The weight-load prologue
        # (g + first bf16 facts chunk first — the first matmul group's deps —
        # then the remaining weights, then the fp8 chunk) sits inside the
        # reps loop so full_body repeat builds re-run it; with wpool bufs=1
        # each rep's weight DMA serializes after the previous rep's last use,
        # mimicking a cold prologue for differential timing.
        for _ in range(reps):
            wt = wpool.tile([128, KT, NW], FP8, tag="wt", name="wt")
            nc.sync.dma_start(wt[:, :, 0:128], wr[:, :, 0:128])
            x8s[0] = x8pool.tile(
                [128, KT, _WIDTHS[0]], FP8, tag="x8", name="x8",
                padded_shape=[128, KT, wmax],
            )
            nc.sync.dma_start(x8s[0][:], f8r[:, :, 0 : _WIDTHS[0]])
            for off in range(128, NW, 384):
                end = min(off + 384, NW)
                nc.sync.dma_start(wt[:, :, off:end], wr[:, :, off:end])
            for mi in range(len(_WIDTHS)):
                x8 = get_xt(mi, x8s, x8pool, FP8, f8r, "x8")
                for n in range(NNW):
                    ps = pspool.tile(
                        [128, _WIDTHS[mi]], F32, tag="ps", name="ps",
                        padded_shape=[128, wmax],
                    )
                    for t in range(KT // 2):
                        nc.tensor.matmul(
                            ps[:],
                            wt[:, 2 * t : 2 * t + 2, n * 128 : (n + 1) * 128],
                            x8[:, 2 * t : 2 * t + 2, :],
                            start=(t == 0),
                            stop=(t == KT // 2 - 1),
                            perf_mode=DR,
                        )
                    drain(ps, pac_b, n * 128, 128, mi)
            x8s.clear()
    return _legalize_sync(nc)


def _shard_inputs(facts, question, Wq, Wa, Wc):
    fx = np.ascontiguousarray(facts, dtype=np.float32).reshape(BL, V)
    fx8 = (fx - np.float32(0.5)).astype(NP_FP8)
    qx = np.asarray(question, dtype=np.float32).sum(axis=1)  # (B, V) bag-of-words
    Wq = np.asarray(Wq, dtype=np.float32)
    Wa = np.asarray(Wa, dtype=np.float32)
    Wc = np.asarray(Wc, dtype=np.float32)
    u0 = qx @ Wq                                  # (B, D) exact, on host
    # match0 = facts @ (Wa0 @ u0^T), (b,b) diagonal only: 64 GEMVs on the
    # host (64 MFLOP via associativity) replace a device group that would
    # compute the full (64, 3200) product, 64x redundant.
    g = Wa[0] @ u0.T                              # (V, B) fp32
    match0 = np.empty((B, L), np.float32)
    for b in range(B):
        match0[b] = fx[b * L : (b + 1) * L] @ g[:, b]
    w8 = (
        np.concatenate([Wa[1], Wa[2], Wc[0], Wc[1], Wc[2]], axis=1)
        * np.float32(WSC)
    ).astype(NP_FP8)

    in_maps = []
    for c in range(NCORES):
        sl = slice(c * VSH, (c + 1) * VSH)
        f8 = np.zeros((VPAD, BL), NP_FP8)
        f8[:VSH] = fx8[:, sl].T
        wb = np.zeros((VPAD, NW), NP_FP8)
        wb[:VSH] = w8[sl]
        in_maps.append({"facts_8": f8, "w_8": wb})
    return in_maps, u0, match0


def _wait_for_devices(min_wait_attempts=10):
    """The axon terminal occasionally reports a transient bad topology
    ("terminal has 1 core"); poll until all 8 NeuronCores are visible."""
    import time as _time

    import jax

    for attempt in range(min_wait_attempts):
        try:
            if len(jax.devices()) >= NCORES:
                return
        except Exception:  # noqa: BLE001 - backend init failure is retryable
            try:
                jax.clear_backends()
            except Exception:  # noqa: BLE001
                pass
        _time.sleep(15.0)
    # fall through: let the run itself raise a descriptive error


def _run_with_retries(nc, in_maps, attempts=4):
    """run_bass_kernel_spmd with retries: the axon terminal occasionally
    reports transient failures (device wedged / NRT_EXEC_UNIT_UNRECOVERABLE /
    temporary topology glitches) that succeed on re-dispatch."""
    import time as _time

    last_exc = None
    for attempt in range(attempts):
        try:
            return run_bass_kernel_spmd(nc, in_maps, list(range(NCORES)))
        except Exception as e:  # noqa: BLE001 - retry any runtime failure
            last_exc = e
            if attempt < attempts - 1:
                _time.sleep(10.0 * (attempt + 1))
                _wait_for_devices(min_wait_attempts=4)
    raise last_exc


def kernel(facts, question, Wq, Wa, Wc, Ww, bw):
    global _nc_cache, _last_result
    _wait_for_devices(min_wait_attempts=8)
    in_maps, u0, match0 = _shard_inputs(facts, question, Wq, Wa, Wc)
    if _nc_cache is None:
        _nc_cache = _build()
    _last_result = _run_with_retries(_nc_cache, in_maps)
    res = _last_result.results

    # Unshard: sum the 8 bf16 partial products of the vocab-sharded matmul.
    ac = res[0]["pac_b"].astype(np.float32)
    for r in res[1:]:
        ac += r["pac_b"].astype(np.float32)

    Wa = np.asarray(Wa, dtype=np.float32)
    Wc = np.asarray(Wc, dtype=np.float32)
    colsum_wa = Wa.sum(axis=1)  # (HOPS, D): exact rank-1 shift corrections
    colsum_wc = Wc.sum(axis=1)

    # Sequential hop recurrence (tiny: ~30 MFLOP vs 98.3 GFLOP on device).
    Ww = np.asarray(Ww, dtype=np.float32)
    bw = np.asarray(bw, dtype=np.float32)
    u = u0
    for h in range(HOPS):
        C = ac[NA + h * D : NA + (h + 1) * D].reshape(D, B, L)  # scaled x2^11
        if h == 0:
            match = match0
        else:
            # A partials carry the 2^11 pre-scale and mean-shifted facts;
            # undo the scale and add the exact rank-1 correction.
            A = ac[(h - 1) * D : h * D].reshape(D, B, L)
            match = np.einsum("dbl,bd->bl", A, u) * np.float32(1.0 / WSC)
            match += np.float32(0.5) * (u @ colsum_wa[h])[:, None]
        mm = match - match.max(axis=-1, keepdims=True)
        e = np.exp(mm)
        p = e / e.sum(axis=-1, keepdims=True)
        # C partials carry the 2^11 fp8 pre-scale; p sums to 1, so the
        # mean-shift correction is a constant vector add.
        att = np.einsum("bl,dbl->bd", p, C) * np.float32(1.0 / WSC)
        att += np.float32(0.5) * colsum_wc[h]
        z = (u + att) @ Ww[h] + bw[h]
        if h == HOPS - 1:
            zz = z - z.max(axis=-1, keepdims=True)
            ez = np.exp(zz)
            u = ez / ez.sum(axis=-1, keepdims=True)
        else:
            u = np.maximum(z, 0.0)
    return np.ascontiguousarray(u, dtype=np.float32)


# revision 20
# speedup vs baseline: 1.1126x; 1.1126x over previous
"""MemNN (end-to-end memory network) Trainium2 kernel.

The heavy FLOPs are six (B*L, V) @ (V, D) embedding matmuls sharing `facts`
as LHS (A_h = facts @ Wa[h], C_h = facts @ Wc[h]) that fuse into one
(3200, 10000) @ (10000, 1280) matmul independent of the hop recurrence.
Sharding: vocab (contraction) split 8 ways; each core computes a partial
product, the host sums the 8 bf16 partials and runs the tiny hop recurrence.

The entire fused matmul runs in fp8 e4m3 with MatmulPerfMode.DoubleRow:
measured on this hardware, DoubleRow streams 2 pair-rows/cycle (4 effective
moving rows/cycle, 2x bf16) and covers two 128-row contraction tiles per
weight load.  One (V, 1280) weight table [Wa1|Wa2|Wc0|Wc1|Wc2], one shifted
fp8 facts tensor, 50 identical DoubleRow instructions per moving chunk.

Precision engineering against the 2e-2 tolerance (measured 1.28e-2):
 - facts are mean-shifted (f - 0.5) before quantizing, halving fp8 noise;
   the exact rank-1 corrections are applied on the host
   (match_h += 0.5*(u . colsum(Wa_h)); att_h += 0.5*colsum(Wc_h), a
   constant add since p sums to 1).  Weights are pre-scaled by 2^11 so
   their ~0.02 entries land in e4m3's normal range (max finite 240).
 - hop 0, whose attention logits have the largest spread and dominate the
   flip risk, is EXACT: u_0 = sum(question) @ Wq and
   match_0[b,l] = facts[b,l,:] . (Wa0 @ u_0[b]) are computed on the host
   in fp32 -- 0.7% of the FLOPs by associativity (the rank-64 projection
   avoids the (B*L, V) @ (V, D) hop-0 matmul entirely).  With hop 0 exact,
   single-term fp8 suffices for A_1/A_2 (1.3e-2 vs 2.4e-2 otherwise).
 - Partials are written in bf16 (halves output DMA; adds ~1e-3 noise).

Moving chunks are 512 wide (the PSUM-bank limit), minimizing instruction
and weight-load count per streamed row.
"""

import os

os.environ.setdefault("MYCRO_LOCAL_CACHE", "1")

import ml_dtypes
import numpy as np

import concourse.bass as bass
import concourse.mybir as mybir
import concourse.tile as tile
from concourse.bass_utils import run_bass_kernel_spmd

HOPS, B, L, V, D = 3, 64, 50, 10000, 256
NCORES = 8
BL = B * L                # 3200 moving rows
NA = 2 * D                # 512 A cols: [Wa1|Wa2]
NC_ = HOPS * D            # 768 C cols: [Wc0|Wc1|Wc2]
NW = NA + NC_             # 1280 fused fp8 weight cols: [Wa1|Wa2|Wc0|Wc1|Wc2]
VSH = V // NCORES         # 1250 vocab rows per core
KT = 10                   # contraction tiles of 128 per core
VPAD = KT * 128           # 1280 (zero-padded)
MCH = 400                 # moving-col chunk
WSC = 2048.0              # 2^11 Wc pre-scale for fp8
F32 = mybir.dt.float32
BF16 = mybir.dt.bfloat16
FP8 = mybir.dt.float8e4
NP_BF16 = ml_dtypes.bfloat16
NP_FP8 = ml_dtypes.float8_e4m3
DR = mybir.MatmulPerfMode.DoubleRow

_nc_cache = None
_last_result = None       # BassKernelResults of the most recent run (for profiling)


def _legalize_sync(nc):
    """Split multi-wait sync_info into standalone single-wait EventSemaphores.

    The walrus build in this environment enforces the raw-bass contract of at
    most ONE SyncWait per instruction ("Too many sync wait commands" in
    setupSyncWait otherwise), while Tile attaches every needed wait to the
    consuming instruction.  Hoisting all-but-one wait onto preceding
    InstEventSemaphore instructions on the same engine queue is semantically
    identical: engine queues are in-order, so a preceding wait blocks the
    queue exactly like an attached wait.  Updates are left untouched (they
    fire at completion and cannot be hoisted).
    """
    for func in nc.m.functions:
        for block in func.blocks:
            insts = list(block.instructions)
            out = []
            n = 0
            for inst in insts:
                si = inst.sync_info
                if si is not None and len(si.on_wait) > 1:
                    waits = list(si.on_wait)
                    for w in waits[:-1]:
                        ev = mybir.InstEventSemaphore(
                            name=f"{inst.name}-hoistw{n}", ins=[], outs=[]
                        )
                        n += 1
                        ev.engine = inst.engine
                        ev.sync_info = mybir.SyncInfo(on_wait=[w], on_update=[])
                        nc.register_instruction(ev)
                        out.append(ev)
                    inst.sync_info = mybir.SyncInfo(
                        on_wait=[waits[-1]], on_update=list(si.on_update)
                    )
                out.append(inst)
            if len(out) != len(insts):
                block.instructions = out
    return nc


# 512-wide chunks (the PSUM-bank / moving-AP limit) minimize instruction
# and weight-load count per streamed row; the 128 tail carries the rest.
_WIDTHS = [512] * 6 + [128]
_STARTS = [sum(_WIDTHS[:i]) for i in range(len(_WIDTHS))]
assert sum(_WIDTHS) == BL


def _build(reps=1):
    """Build the SPMD device program.

    reps>1 repeats the main loop body (same data, same output addresses) --
    used only by the benchmark harness to measure device time differentially
    (per-call dispatch noise over the axon tunnel is ~ms, device time is
    ~100 us, so wall-clocking one launch cannot resolve it).
    """
    nc = bass.Bass(trn_type="TRN2")
    facts_8 = nc.dram_tensor("facts_8", [VPAD, BL], FP8, kind="ExternalInput")
    w_8 = nc.dram_tensor("w_8", [VPAD, NW], FP8, kind="ExternalInput")
    pac_b = nc.dram_tensor("pac_b", [NW, BL], BF16, kind="ExternalOutput")

    f8r = facts_8.rearrange("(k p) n -> p k n", p=128)
    wr = w_8.rearrange("(k p) n -> p k n", p=128)
    wmax = max(_WIDTHS)
    NNW = NW // 128           # 10 fp8 DoubleRow n-tiles

    with (
        tile.TileContext(nc) as tc,
        tc.tile_pool(name="wpool", bufs=1) as wpool,
        tc.tile_pool(name="x8pool", bufs=3) as x8pool,
        tc.tile_pool(name="opool", bufs=6) as opool,
        tc.tile_pool(name="pspool", bufs=7, space="PSUM") as pspool,
    ):
        x8s = {}

        def get_xt(mi, xs, pool, dt, rr, tg):
            if mi not in xs:
                xs[mi] = pool.tile(
                    [128, KT, _WIDTHS[mi]], dt, tag=tg, name=tg,
                    padded_shape=[128, KT, wmax],
                )
                nc.sync.dma_start(
                    xs[mi][:], rr[:, :, _STARTS[mi] : _STARTS[mi] + _WIDTHS[mi]]
                )
            return xs[mi]

        def drain(ps, dst, row0, nrows, mi):
            ot = opool.tile(
                [nrows, _WIDTHS[mi]], BF16, tag="ot", name="ot",
                padded_shape=[128, wmax],
            )
            nc.vector.tensor_copy(ot[:], ps[:])
            nc.sync.dma_start(
                dst[row0 : row0 + nrows, _STARTS[mi] : _STARTS[mi] + _WIDTHS[mi]],
                ot[:],
            )

        # Main fused matmul per chunk: the 64-wide match0 group, 4 bf16 A
        # n-tiles, then 6 fp8 DoubleRow C n-tiles.  The weight-load prologue
        # (g + first bf16 facts chunk first — the first matmul group's deps —
        # then the remaining weights, then the fp8 chunk) sits inside the
        # reps loop so full_body repeat builds re-run it; with wpool bufs=1
        # each rep's weight DMA serializes after the previous rep's last use,
        # mimicking a cold prologue for differential timing.
        for _ in range(reps):
            wt = wpool.tile([128, KT, NW], FP8, tag="wt", name="wt")
            nc.sync.dma_start(wt[:, :, 0:128], wr[:, :, 0:128])
            x8s[0] = x8pool.tile(
                [128, KT, _WIDTHS[0]], FP8, tag="x8", name="x8",
                padded_shape=[128, KT, wmax],
            )
            nc.sync.dma_start(x8s[0][:], f8r[:, :, 0 : _WIDTHS[0]])
            for off in range(128, NW, 384):
                end = min(off + 384, NW)
                nc.sync.dma_start(wt[:, :, off:end], wr[:, :, off:end])
            for mi in range(len(_WIDTHS)):
                x8 = get_xt(mi, x8s, x8pool, FP8, f8r, "x8")
                for n in range(NNW):
                    ps = pspool.tile(
                        [128, _WIDTHS[mi]], F32, tag="ps", name="ps",
                        padded_shape=[128, wmax],
                    )
                    for t in range(KT // 2):
                        nc.tensor.matmul(
                            ps[:],
                            wt[:, 2 * t : 2 * t + 2, n * 128 : (n + 1) * 128],
                            x8[:, 2 * t : 2 * t + 2, :],
                            start=(t == 0),
                            stop=(t == KT // 2 - 1),
                            perf_mode=DR,
                        )
                    drain(ps, pac_b, n * 128, 128, mi)
            x8s.clear()
    return _legalize_sync(nc)


def _shard_inputs(facts, question, Wq, Wa, Wc):
    fx = np.ascontiguousarray(facts, dtype=np.float32).reshape(BL, V)
    fx8 = (fx - np.float32(0.5)).astype(NP_FP8)
    qx = np.asarray(question, dtype=np.float32).sum(axis=1)  # (B, V) bag-of-words
    Wq = np.asarray(Wq, dtype=np.float32)
    Wa = np.asarray(Wa, dtype=np.float32)
    Wc = np.asarray(Wc, dtype=np.float32)
    u0 = qx @ Wq                                  # (B, D) exact, on host
    # match0 = facts @ (Wa0 @ u0^T), (b,b) diagonal only: 64 GEMVs on the
    # host (64 MFLOP via associativity) replace a device group that would
    # compute the full (64, 3200) product, 64x redundant.
    g = Wa[0] @ u0.T                              # (V, B) fp32
    match0 = np.empty((B, L), np.float32)
    for b in range(B):
        match0[b] = fx[b * L : (b + 1) * L] @ g[:, b]
    w8 = (
        np.concatenate([Wa[1], Wa[2], Wc[0], Wc[1], Wc[2]], axis=1)
        * np.float32(WSC)
    ).astype(NP_FP8)

    in_maps = []
    for c in range(NCORES):
        sl = slice(c * VSH, (c + 1) * VSH)
        f8 = np.zeros((VPAD, BL), NP_FP8)
        f8[:VSH] = fx8[:, sl].T
        wb = np.zeros((VPAD, NW), NP_FP8)
        wb[:VSH] = w8[sl]
        in_maps.append({"facts_8": f8, "w_8": wb})
    return in_maps, u0, match0


def _wait_for_devices(min_wait_attempts=10):
    """The axon terminal occasionally reports a transient bad topology
    ("terminal has 1 core"); poll until all 8 NeuronCores are visible."""
    import time as _time

    import jax

    for attempt in range(min_wait_attempts):
        try:
            if len(jax.devices()) >= NCORES:
                return
        except Exception:  # noqa: BLE001 - backend init failure is retryable
            try:
                jax.clear_backends()
            except Exception:  # noqa: BLE001
                pass
        _time.sleep(15.0)
    # fall through: let the run itself raise a descriptive error


def _run_with_retries(nc, in_maps, attempts=4):
    """run_bass_kernel_spmd with retries: the axon terminal occasionally
    reports transient failures (device wedged / NRT_EXEC_UNIT_UNRECOVERABLE /
    temporary topology glitches) that succeed on re-dispatch."""
    import time as _time

    last_exc = None
    for attempt in range(attempts):
        try:
            return run_bass_kernel_spmd(nc, in_maps, list(range(NCORES)))
        except Exception as e:  # noqa: BLE001 - retry any runtime failure
            last_exc = e
            if attempt < attempts - 1:
                _time.sleep(10.0 * (attempt + 1))
                _wait_for_devices(min_wait_attempts=4)
    raise last_exc


def kernel(facts, question, Wq, Wa, Wc, Ww, bw):
    global _nc_cache, _last_result
    _wait_for_devices(min_wait_attempts=8)
    in_maps, u0, match0 = _shard_inputs(facts, question, Wq, Wa, Wc)
    if _nc_cache is None:
        _nc_cache = _build()
    _last_result = _run_with_retries(_nc_cache, in_maps)
    res = _last_result.results

    # Unshard: sum the 8 bf16 partial products of the vocab-sharded matmul.
    ac = res[0]["pac_b"].astype(np.float32)
    for r in res[1:]:
        ac += r["pac_b"].astype(np.float32)

    Wa = np.asarray(Wa, dtype=np.float32)
    Wc = np.asarray(Wc, dtype=np.float32)
    colsum_wa = Wa.sum(axis=1)  # (HOPS, D): exact rank-1 shift corrections
    colsum_wc = Wc.sum(axis=1)

    # Sequential hop recurrence (tiny: ~30 MFLOP vs 98.3 GFLOP on device).
    Ww = np.asarray(Ww, dtype=np.float32)
    bw = np.asarray(bw, dtype=np.float32)
    u = u0
    for h in range(HOPS):
        C = ac[NA + h * D : NA + (h + 1) * D].reshape(D, B, L)  # scaled x2^11
        if h == 0:
            match = match0
        else:
            # A partials carry the 2^11 pre-scale and mean-shifted facts;
            # undo the scale and add the exact rank-1 correction.
            A = ac[(h - 1) * D : h * D].reshape(D, B, L)
            match = np.einsum("dbl,bd->bl", A, u) * np.float32(1.0 / WSC)
            match += np.float32(0.5) * (u @ colsum_wa[h])[:, None]
        mm = match - match.max(axis=-1, keepdims=True)
        e = np.exp(mm)
        p = e / e.sum(axis=-1, keepdims=True)
        # C partials carry the 2^11 fp8 pre-scale; p sums to 1, so the
        # mean-shift correction is a constant vector add.
        att = np.einsum("bl,dbl->bd", p, C) * np.float32(1.0 / WSC)
        att += np.float32(0.5) * colsum_wc[h]
        z = (u + att) @ Ww[h] + bw[h]
        if h == HOPS - 1:
            zz = z - z.max(axis=-1, keepdims=True)
            ez = np.exp(zz)
            u = ez / ez.sum(axis=-1, keepdims=True)
        else:
            u = np.maximum(z, 0.0)
    return np.ascontiguousarray(u, dtype=np.float32)
